# revision 19
# baseline (speedup 1.0000x reference)
"""Trainium2 Bass kernel for an RNN-T style joint network MLP.

  out[b,t,u,o] = tanh(enc[b,t,:] @ W1[:512] + dec[b,u,:] @ W1[512:] + b1) @ W2 + b2

Shapes: enc (8, 256, 512), dec (8, 64, 512), W1 (1024, 1024), b1 (1024,),
W2 (1024, 128), b2 (128,), out (8, 256, 64, 128), all float32.

Sharding: data-parallel over batch - one batch element per NeuronCore, no
collectives. fp16 datapath (fp32 PSUM accumulation).

Per core, the elementwise stage (16.8M elements) is split across engines:
  - ACT: wide tanh for most u-columns (~0.87 ns/col measured).
  - DVE: narrow per-(hc,u) tensor_scalar adds build sum = ep + (dp[u]+b1)
    for ALL u (4x mode), plus a polynomial-tanh pipeline for UC u's per
    superblock: TS dual (max,mult alpha) -> custom 8-stage DVE op (monic
    deg-9 odd Horner) -> tensor_tensor mult. The custom op evaluates
    m(y)=(((y+a3)y+a2)y+a1)y+a0 in ONE DVE pass (1 elem/cycle/lane).
  - PE: e_proj/dec_proj head GEMMs, then main GEMM per superblock of
    16 u (8 pairs x 8 psum banks), hc-outer so W2 chunks stay warm.
  - PSUM evacuation (+b2) alternates ACT/DVE per pair.
"""

import os
import numpy as np
import ml_dtypes

B, T, U, D, H, O = 8, 256, 64, 512, 1024, 128
NCORES = 8
HC = H // 128      # 8 h-chunks
SB = 16            # u per superblock
NSB = U // SB      # 4 superblocks
NPAIR = SB // 2    # 8 pairs per superblock (one psum bank each)
UC = int(os.environ.get("KERNEL_UC", "0"))  # poly-route u's per superblock
SBW = SB * T       # sum-tile width per (sb, hc)
HW = SBW // 2      # half-superblock psum width (4 banks)

# tanh(x) ~= m(y')*(alpha*xc), xc = clamp(x, +-R), y' = (alpha*xc)^2,
# m(y) = (((y + A3)y + A2)y + A1)y + A0  (see /tmp/poly9 fit, max err 5.4e-3)
R = 3.0
ALPHA = 0.3866979708437265
A3 = -3.825905079725885
A2 = 5.677982486954507
A1 = -4.3838240511686255
A0 = 2.523557764197638

_CACHE = {}
LAST_RESULT = None


def _register_horner9():
    """Custom DVE op: out = (((y + s0)*y + s1)*y + imm2)*y + in1, y = in0^2."""
    import concourse.dve_ops as dve_ops
    from concourse.dve_spec import (
        Spec, Src0, C0, C1, C2, C3, sq, lower, _spill_c3_to_src1, _has_src1,
    )
    from concourse.dve_uop import DveOpSpec

    name = "TANH_POLY9_ANT"
    if name in dve_ops._SUB_OPCODE_FOR_NAME:
        return next(o for o in dve_ops.OPS if o.name == name)
    y = sq(Src0)
    body = _spill_c3_to_src1((((y + C0) * y + C1) * y + C2) * y + C3)

    def ref(in0, in1, c0, c1, c2):
        yy = in0.astype(np.float32) ** 2
        return (((yy + c0) * yy + c1) * yy + c2) * yy + in1

    spec = Spec(body=body, reference=ref)
    row = dve_ops._CUSTOM_DVE_ROW_BASE + len(dve_ops.OPS)
    shas = {}
    for ver in ("v3", "v4"):
        uops = lower(spec, ver=ver)
        shas[ver] = DveOpSpec(name=name, opcode=row, uops=uops,
                              rd1_en=_has_src1(spec)).sha(ver)
    op = dve_ops.DveOp(name, spec, subdim=False, uops_sha=shas)
    dve_ops.OPS.append(op)
    dve_ops.CUSTOM_DVE_SPECS[name] = spec
    dve_ops._SUB_OPCODE_FOR_NAME[name] = row
    return op


def _build_program():
    from concourse import bacc, tile
    import concourse.mybir as mybir

    OP9 = _register_horner9()
    dt = mybir.dt
    f32, f16 = dt.float32, dt.float16
    Act = mybir.ActivationFunctionType
    Alu = mybir.AluOpType

    nc = bacc.Bacc("TRN2", target_bir_lowering=False, debug=False)

    encT = nc.dram_tensor("encT", [D, T], f16, kind="ExternalInput").ap()
    decT = nc.dram_tensor("decT", [D, U], f16, kind="ExternalInput").ap()
    W1 = nc.dram_tensor("W1", [2 * D, H], f16, kind="ExternalInput").ap()
    W2h = nc.dram_tensor("W2h", [H, O], f16, kind="ExternalInput").ap()
    b1r = nc.dram_tensor("b1r", [128, HC], f32, kind="ExternalInput").ap()
    b2c = nc.dram_tensor("b2c", [O, 1], f32, kind="ExternalInput").ap()
    outT = nc.dram_tensor("outT", [O, U, T], f32, kind="ExternalOutput").ap()

    with tile.TileContext(nc) as tc:
        with tc.tile_pool(name="persist", bufs=1) as persist, \
             tc.tile_pool(name="sums", bufs=3) as sums_pool, \
             tc.tile_pool(name="axcp", bufs=3) as axc_pool, \
             tc.tile_pool(name="qp", bufs=3) as q_pool, \
             tc.tile_pool(name="tanhp", bufs=2) as tanh_pool, \
             tc.tile_pool(name="outsb", bufs=3) as out_pool, \
             tc.tile_pool(name="psum", bufs=2, space="PSUM") as psum_pool:

            w1_sb = persist.tile([128, 8 * H], f16, tag="w1")
            encT_sb = persist.tile([128, 4 * T], f16, tag="encT")
            decT_sb = persist.tile([128, 4 * U], f16, tag="decT")
            w2_sb = persist.tile([128, HC * O], f16, tag="w2")
            b1_sb = persist.tile([128, HC], f32, tag="b1")
            b2_sb = persist.tile([128, 1], f32, tag="b2")
            a0_sb = persist.tile([128, 1], f32, tag="a0")
            e_sb = persist.tile([128, HC * T], f16, tag="eproj")
            bias_sb = persist.tile([128, HC * U], f32, tag="bias")

            nc.vector.memset(a0_sb[:], A0)

            # ---- input loads. W1 arrives in per-hc column slices so head
            # GEMM hc can start as soon as its own 256KB lands.
            nc.sync.dma_start(
                encT_sb[:, :].rearrange("p (c t) -> p c t", c=4),
                encT[:, :].rearrange("(c p) t -> p c t", p=128))
            nc.sync.dma_start(
                decT_sb[:, :].rearrange("p (c u) -> p c u", c=4),
                decT[:, :].rearrange("(c p) u -> p c u", p=128))
            nc.sync.dma_start(b1_sb[:], b1r[:, :])
            nc.sync.dma_start(b2_sb[:], b2c[:, :])
            w1v_e = w1_sb[:, 0:4 * H].rearrange("p (c h) -> p c h", c=4)
            w1v_d = w1_sb[:, 4 * H:8 * H].rearrange("p (c h) -> p c h", c=4)
            for hc in range(HC):
                # W1 cols for this hc: [512, 128] -> [128, 4, 128]
                nc.sync.dma_start(
                    w1v_e[:, :, hc * 128:(hc + 1) * 128],
                    W1[0:512, hc * 128:(hc + 1) * 128]
                    .rearrange("(c p) h -> p c h", p=128))
                nc.sync.dma_start(
                    w1v_d[:, :, hc * 128:(hc + 1) * 128],
                    W1[512:1024, hc * 128:(hc + 1) * 128]
                    .rearrange("(c p) h -> p c h", p=128))
            nc.sync.dma_start(
                w2_sb[:, :].rearrange("p (c o) -> p c o", c=HC),
                W2h[:, :].rearrange("(c p) o -> p c o", p=128))

            # ---- head GEMMs per h-chunk
            # e_projT[h,t] = sum_d W_enc[d,h] encT[d,t]       -> e_sb (f16)
            # bias[h,u]    = sum_d W_dec[d,h] decT[d,u] + b1  -> bias_sb (f32)
            for hc in range(HC):
                pe = psum_pool.tile([128, T], f32, tag="ps", name=f"pe{hc}")
                for dc in range(4):
                    nc.tensor.matmul(
                        pe[:],
                        lhsT=w1_sb[:, dc * H + hc * 128: dc * H + hc * 128 + 128],
                        rhs=encT_sb[:, dc * T:(dc + 1) * T],
                        start=(dc == 0), stop=(dc == 3),
                    )
                nc.scalar.activation(e_sb[:, hc * T:(hc + 1) * T], pe[:],
                                     Act.Identity)

                pd = psum_pool.tile([128, U], f32, tag="ps", name=f"pd{hc}")
                for dc in range(4):
                    nc.tensor.matmul(
                        pd[:],
                        lhsT=w1_sb[:, (4 + dc) * H + hc * 128: (4 + dc) * H + hc * 128 + 128],
                        rhs=decT_sb[:, dc * U:(dc + 1) * U],
                        start=(dc == 0), stop=(dc == 3),
                    )
                nc.vector.tensor_scalar_add(bias_sb[:, hc * U:(hc + 1) * U],
                                            pd[:], b1_sb[:, hc:hc + 1])

            # ---- steady pipeline over superblocks of 16 u ----
            # tanh tile layout per sb: [hc][u_local][t]; u_local 0..UC-1 are
            # the DVE-poly route, the rest the ACT route.
            UCW = UC * T

            def emit_evac_chunk(sb, pos, k, nev, on_act):
                # one psum evac chunk (+b2 as per-partition bias) + its DMA
                w = 2 * HW // nev
                half, ev = divmod(k, nev // 2)
                osb = out_pool.tile([128, w], f32, tag="osb",
                                    name=f"ev{sb}_{k}")
                src = pos[half][:, ev * w:(ev + 1) * w]
                if on_act:
                    nc.scalar.activation(osb[:], src, Act.Identity,
                                         bias=b2_sb[:, 0:1])
                else:
                    nc.vector.tensor_scalar_add(osb[:], src, b2_sb[:, 0:1])
                nu = SB // nev
                u0 = sb * SB + k * nu
                nc.sync.dma_start(outT[:, u0:u0 + nu, :], osb[:])

            pending = None
            for sb in range(NSB):
                tanh_sb = tanh_pool.tile([128, HC * SBW], f16, tag="tanh")
                pos = [psum_pool.tile([128, HW], f32, tag="ps",
                                      name=f"po{sb}_{h}") for h in range(2)]
                for hc in range(HC):
                    sum_sb = sums_pool.tile([128, SBW], f16, tag="sum")
                    hoff = hc * SBW
                    # adds: sum[u,t] = ep[hc] + bias[hc,u]; clamp-high for
                    # the poly u's rides the dual-op slot.
                    for ul in range(SB):
                        u = sb * SB + ul
                        dst = sum_sb[:, ul * T:(ul + 1) * T]
                        src = e_sb[:, hc * T:(hc + 1) * T]
                        bu = bias_sb[:, hc * U + u: hc * U + u + 1]
                        if ul < UC:
                            nc.vector.tensor_scalar(dst, src, bu, R,
                                                    Alu.add, Alu.min)
                        else:
                            nc.vector.tensor_scalar_add(dst, src, bu)

                    # ACT route: one wide tanh per (sb, hc)
                    nc.scalar.activation(
                        tanh_sb[:, hoff + UCW: hoff + SBW],
                        sum_sb[:, UCW:SBW], Act.Tanh)

                    # DVE poly route
                    if UC:
                        axc = axc_pool.tile([128, UCW], f16, tag="axc")
                        qt = q_pool.tile([128, UCW], f16, tag="q")
                        nc.vector.tensor_scalar(axc[:], sum_sb[:, 0:UCW],
                                                -R, ALPHA, Alu.max, Alu.mult)
                        nc.vector._custom_dve(OP9, out=qt[:], in0=axc[:],
                                              in1=a0_sb[:, 0:1],
                                              s0=A3, s1=A2, imm2=A1)
                        nc.vector.tensor_tensor(tanh_sb[:, hoff: hoff + UCW],
                                                qt[:], axc[:], Alu.mult)

                    # main GEMM for this hc (W2 chunk stays warm)
                    for p in range(NPAIR):
                        nc.tensor.matmul(
                            pos[p // 4][:, (p % 4) * 2 * T:(p % 4 + 1) * 2 * T],
                            lhsT=w2_sb[:, hc * O:(hc + 1) * O],
                            rhs=tanh_sb[:, hoff + p * 2 * T: hoff + (p + 1) * 2 * T],
                            start=(hc == 0), stop=(hc == HC - 1),
                        )
                    # previous superblock's evacs, interleaved into this
                    # superblock's stream. Half 0 fully at hc==0 and half 1
                    # at hc==1, so each psum slot frees just before this
                    # superblock's accumulation needs it, and the strict-FIFO
                    # engine queues never stall on the PE finishing.
                    if pending is not None and hc < 2:
                        emit_evac_chunk(sb - 1, pending, 2 * hc, 4,
                                        on_act=False)
                        emit_evac_chunk(sb - 1, pending, 2 * hc + 1, 4,
                                        on_act=False)
                pending = pos
            # drain: last superblock in fine chunks, alternating engines
            for k in range(8):
                emit_evac_chunk(NSB - 1, pending, k, 8, on_act=(k % 2 == 1))

    nc.compile()
    return nc


def kernel(encoder_state, decoder_state, W1, b1, W2, b2):
    from concourse.bass_utils import run_bass_kernel_spmd
    global LAST_RESULT

    if "nc" not in _CACHE:
        _CACHE["nc"] = _build_program()
    nc = _CACHE["nc"]

    encoder_state = np.asarray(encoder_state, dtype=np.float32)
    decoder_state = np.asarray(decoder_state, dtype=np.float32)
    W1 = np.asarray(W1, dtype=np.float32)
    b1 = np.asarray(b1, dtype=np.float32)
    W2 = np.asarray(W2, dtype=np.float32)
    b2 = np.asarray(b2, dtype=np.float32)

    h16 = np.float16
    W1h = W1.astype(h16)
    W2h = W2.astype(h16)
    b1r = np.ascontiguousarray(b1.reshape(HC, 128).T)  # [128, 8]
    b2c = np.ascontiguousarray(b2.reshape(O, 1))

    in_maps = []
    for i in range(NCORES):
        in_maps.append({
            "encT": np.ascontiguousarray(encoder_state[i].T.astype(h16)),
            "decT": np.ascontiguousarray(decoder_state[i].T.astype(h16)),
            "W1": W1h,
            "W2h": W2h,
            "b1r": b1r,
            "b2c": b2c,
        })

    trace = bool(int(os.environ.get("KERNEL_TRACE", "0")))
    res = run_bass_kernel_spmd(nc, in_maps, list(range(NCORES)), trace=trace)
    LAST_RESULT = res

    out = np.empty((B, T, U, O), dtype=np.float32)
    for i in range(NCORES):
        out[i] = res.results[i]["outT"].transpose(2, 1, 0)
    return out


# revision 20
# speedup vs baseline: 1.0060x; 1.0060x over previous
"""Trainium2 Bass kernel for an RNN-T style joint network MLP.

  out[b,t,u,o] = tanh(enc[b,t,:] @ W1[:512] + dec[b,u,:] @ W1[512:] + b1) @ W2 + b2

Shapes: enc (8, 256, 512), dec (8, 64, 512), W1 (1024, 1024), b1 (1024,),
W2 (1024, 128), b2 (128,), out (8, 256, 64, 128), all float32.

Sharding: data-parallel over batch - one batch element per NeuronCore, no
collectives. fp16 datapath (fp32 PSUM accumulation).

Per core, the elementwise stage (16.8M elements) is split across engines:
  - ACT: wide tanh for most u-columns (~0.87 ns/col measured).
  - DVE: narrow per-(hc,u) tensor_scalar adds build sum = ep + (dp[u]+b1)
    for ALL u (4x mode), plus a polynomial-tanh pipeline for UC u's per
    superblock: TS dual (max,mult alpha) -> custom 8-stage DVE op (monic
    deg-9 odd Horner) -> tensor_tensor mult. The custom op evaluates
    m(y)=(((y+a3)y+a2)y+a1)y+a0 in ONE DVE pass (1 elem/cycle/lane).
  - PE: e_proj/dec_proj head GEMMs, then main GEMM per superblock of
    16 u (8 pairs x 8 psum banks), hc-outer so W2 chunks stay warm.
  - PSUM evacuation (+b2) alternates ACT/DVE per pair.
"""

import os
import numpy as np
import ml_dtypes

B, T, U, D, H, O = 8, 256, 64, 512, 1024, 128
NCORES = 8
HC = H // 128      # 8 h-chunks
SB = 16            # u per superblock
NSB = U // SB      # 4 superblocks
NPAIR = SB // 2    # 8 pairs per superblock (one psum bank each)
UC = int(os.environ.get("KERNEL_UC", "0"))  # poly-route u's per superblock
SBW = SB * T       # sum-tile width per (sb, hc)
HW = SBW // 2      # half-superblock psum width (4 banks)

# tanh(x) ~= m(y')*(alpha*xc), xc = clamp(x, +-R), y' = (alpha*xc)^2,
# m(y) = (((y + A3)y + A2)y + A1)y + A0  (see /tmp/poly9 fit, max err 5.4e-3)
R = 3.0
ALPHA = 0.3866979708437265
A3 = -3.825905079725885
A2 = 5.677982486954507
A1 = -4.3838240511686255
A0 = 2.523557764197638

_CACHE = {}
LAST_RESULT = None


def _register_horner9():
    """Custom DVE op: out = (((y + s0)*y + s1)*y + imm2)*y + in1, y = in0^2."""
    import concourse.dve_ops as dve_ops
    from concourse.dve_spec import (
        Spec, Src0, C0, C1, C2, C3, sq, lower, _spill_c3_to_src1, _has_src1,
    )
    from concourse.dve_uop import DveOpSpec

    name = "TANH_POLY9_ANT"
    if name in dve_ops._SUB_OPCODE_FOR_NAME:
        return next(o for o in dve_ops.OPS if o.name == name)
    y = sq(Src0)
    body = _spill_c3_to_src1((((y + C0) * y + C1) * y + C2) * y + C3)

    def ref(in0, in1, c0, c1, c2):
        yy = in0.astype(np.float32) ** 2
        return (((yy + c0) * yy + c1) * yy + c2) * yy + in1

    spec = Spec(body=body, reference=ref)
    row = dve_ops._CUSTOM_DVE_ROW_BASE + len(dve_ops.OPS)
    shas = {}
    for ver in ("v3", "v4"):
        uops = lower(spec, ver=ver)
        shas[ver] = DveOpSpec(name=name, opcode=row, uops=uops,
                              rd1_en=_has_src1(spec)).sha(ver)
    op = dve_ops.DveOp(name, spec, subdim=False, uops_sha=shas)
    dve_ops.OPS.append(op)
    dve_ops.CUSTOM_DVE_SPECS[name] = spec
    dve_ops._SUB_OPCODE_FOR_NAME[name] = row
    return op


def _build_program():
    from concourse import bacc, tile
    import concourse.mybir as mybir

    OP9 = _register_horner9()
    dt = mybir.dt
    f32, f16 = dt.float32, dt.float16
    Act = mybir.ActivationFunctionType
    Alu = mybir.AluOpType

    nc = bacc.Bacc("TRN2", target_bir_lowering=False, debug=False)

    encT = nc.dram_tensor("encT", [D, T], f16, kind="ExternalInput").ap()
    decT = nc.dram_tensor("decT", [D, U], f16, kind="ExternalInput").ap()
    W1 = nc.dram_tensor("W1", [2 * D, H], f16, kind="ExternalInput").ap()
    W2h = nc.dram_tensor("W2h", [H, O], f16, kind="ExternalInput").ap()
    b1r = nc.dram_tensor("b1r", [128, HC], f32, kind="ExternalInput").ap()
    b2c = nc.dram_tensor("b2c", [O, 1], f32, kind="ExternalInput").ap()
    outT = nc.dram_tensor("outT", [O, U, T], f32, kind="ExternalOutput").ap()

    with tile.TileContext(nc) as tc:
        with tc.tile_pool(name="persist", bufs=1) as persist, \
             tc.tile_pool(name="sums", bufs=3) as sums_pool, \
             tc.tile_pool(name="axcp", bufs=3) as axc_pool, \
             tc.tile_pool(name="qp", bufs=3) as q_pool, \
             tc.tile_pool(name="tanhp", bufs=2) as tanh_pool, \
             tc.tile_pool(name="outsb", bufs=3) as out_pool, \
             tc.tile_pool(name="psum", bufs=2, space="PSUM") as psum_pool:

            w1_sb = persist.tile([128, 8 * H], f16, tag="w1")
            encT_sb = persist.tile([128, 4 * T], f16, tag="encT")
            decT_sb = persist.tile([128, 4 * U], f16, tag="decT")
            w2_sb = persist.tile([128, HC * O], f16, tag="w2")
            b1_sb = persist.tile([128, HC], f32, tag="b1")
            b2_sb = persist.tile([128, 1], f32, tag="b2")
            a0_sb = persist.tile([128, 1], f32, tag="a0")
            e_sb = persist.tile([128, HC * T], f16, tag="eproj")
            bias_sb = persist.tile([128, HC * U], f32, tag="bias")

            nc.vector.memset(a0_sb[:], A0)

            # ---- input loads. W1 arrives in per-hc column slices so head
            # GEMM hc can start as soon as its own 256KB lands.
            nc.sync.dma_start(
                encT_sb[:, :].rearrange("p (c t) -> p c t", c=4),
                encT[:, :].rearrange("(c p) t -> p c t", p=128))
            nc.sync.dma_start(
                decT_sb[:, :].rearrange("p (c u) -> p c u", c=4),
                decT[:, :].rearrange("(c p) u -> p c u", p=128))
            nc.sync.dma_start(b1_sb[:], b1r[:, :])
            nc.sync.dma_start(b2_sb[:], b2c[:, :])
            w1v_e = w1_sb[:, 0:4 * H].rearrange("p (c h) -> p c h", c=4)
            w1v_d = w1_sb[:, 4 * H:8 * H].rearrange("p (c h) -> p c h", c=4)
            for hc in range(HC):
                # W1 cols for this hc: [512, 128] -> [128, 4, 128]
                nc.sync.dma_start(
                    w1v_e[:, :, hc * 128:(hc + 1) * 128],
                    W1[0:512, hc * 128:(hc + 1) * 128]
                    .rearrange("(c p) h -> p c h", p=128))
                nc.sync.dma_start(
                    w1v_d[:, :, hc * 128:(hc + 1) * 128],
                    W1[512:1024, hc * 128:(hc + 1) * 128]
                    .rearrange("(c p) h -> p c h", p=128))
            nc.sync.dma_start(
                w2_sb[:, :].rearrange("p (c o) -> p c o", c=HC),
                W2h[:, :].rearrange("(c p) o -> p c o", p=128))

            # ---- head GEMMs per h-chunk
            # e_projT[h,t] = sum_d W_enc[d,h] encT[d,t]       -> e_sb (f16)
            # bias[h,u]    = sum_d W_dec[d,h] decT[d,u] + b1  -> bias_sb (f32)
            for hc in range(HC):
                pe = psum_pool.tile([128, T], f32, tag="ps", name=f"pe{hc}")
                for dc in range(4):
                    nc.tensor.matmul(
                        pe[:],
                        lhsT=w1_sb[:, dc * H + hc * 128: dc * H + hc * 128 + 128],
                        rhs=encT_sb[:, dc * T:(dc + 1) * T],
                        start=(dc == 0), stop=(dc == 3),
                    )
                nc.scalar.activation(e_sb[:, hc * T:(hc + 1) * T], pe[:],
                                     Act.Identity)

                pd = psum_pool.tile([128, U], f32, tag="ps", name=f"pd{hc}")
                for dc in range(4):
                    nc.tensor.matmul(
                        pd[:],
                        lhsT=w1_sb[:, (4 + dc) * H + hc * 128: (4 + dc) * H + hc * 128 + 128],
                        rhs=decT_sb[:, dc * U:(dc + 1) * U],
                        start=(dc == 0), stop=(dc == 3),
                    )
                nc.scalar.activation(bias_sb[:, hc * U:(hc + 1) * U], pd[:],
                                     Act.Identity, bias=b1_sb[:, hc:hc + 1])

            # ---- steady pipeline over superblocks of 16 u ----
            # tanh tile layout per sb: [hc][u_local][t]; u_local 0..UC-1 are
            # the DVE-poly route, the rest the ACT route.
            UCW = UC * T

            def emit_evac_chunk(sb, pos, k, nev, on_act):
                # one psum evac chunk (+b2 as per-partition bias) + its DMA
                w = 2 * HW // nev
                half, ev = divmod(k, nev // 2)
                osb = out_pool.tile([128, w], f32, tag="osb",
                                    name=f"ev{sb}_{k}")
                src = pos[half][:, ev * w:(ev + 1) * w]
                if on_act:
                    nc.scalar.activation(osb[:], src, Act.Identity,
                                         bias=b2_sb[:, 0:1])
                else:
                    nc.vector.tensor_scalar_add(osb[:], src, b2_sb[:, 0:1])
                nu = SB // nev
                u0 = sb * SB + k * nu
                nc.sync.dma_start(outT[:, u0:u0 + nu, :], osb[:])

            pending = None
            for sb in range(NSB):
                tanh_sb = tanh_pool.tile([128, HC * SBW], f16, tag="tanh")
                pos = [psum_pool.tile([128, HW], f32, tag="ps",
                                      name=f"po{sb}_{h}") for h in range(2)]
                for hc in range(HC):
                    sum_sb = sums_pool.tile([128, SBW], f16, tag="sum")
                    hoff = hc * SBW
                    # adds: sum[u,t] = ep[hc] + bias[hc,u]; clamp-high for
                    # the poly u's rides the dual-op slot.
                    for ul in range(SB):
                        u = sb * SB + ul
                        dst = sum_sb[:, ul * T:(ul + 1) * T]
                        src = e_sb[:, hc * T:(hc + 1) * T]
                        bu = bias_sb[:, hc * U + u: hc * U + u + 1]
                        if ul < UC:
                            nc.vector.tensor_scalar(dst, src, bu, R,
                                                    Alu.add, Alu.min)
                        else:
                            nc.vector.tensor_scalar_add(dst, src, bu)

                    # ACT route: one wide tanh per (sb, hc)
                    nc.scalar.activation(
                        tanh_sb[:, hoff + UCW: hoff + SBW],
                        sum_sb[:, UCW:SBW], Act.Tanh)

                    # DVE poly route
                    if UC:
                        axc = axc_pool.tile([128, UCW], f16, tag="axc")
                        qt = q_pool.tile([128, UCW], f16, tag="q")
                        nc.vector.tensor_scalar(axc[:], sum_sb[:, 0:UCW],
                                                -R, ALPHA, Alu.max, Alu.mult)
                        nc.vector._custom_dve(OP9, out=qt[:], in0=axc[:],
                                              in1=a0_sb[:, 0:1],
                                              s0=A3, s1=A2, imm2=A1)
                        nc.vector.tensor_tensor(tanh_sb[:, hoff: hoff + UCW],
                                                qt[:], axc[:], Alu.mult)

                    # main GEMM for this hc (W2 chunk stays warm)
                    for p in range(NPAIR):
                        nc.tensor.matmul(
                            pos[p // 4][:, (p % 4) * 2 * T:(p % 4 + 1) * 2 * T],
                            lhsT=w2_sb[:, hc * O:(hc + 1) * O],
                            rhs=tanh_sb[:, hoff + p * 2 * T: hoff + (p + 1) * 2 * T],
                            start=(hc == 0), stop=(hc == HC - 1),
                        )
                    # previous superblock's evacs, interleaved into this
                    # superblock's stream. Half 0 fully at hc==0 and half 1
                    # at hc==1, so each psum slot frees just before this
                    # superblock's accumulation needs it, and the strict-FIFO
                    # engine queues never stall on the PE finishing.
                    if pending is not None and hc < 2:
                        emit_evac_chunk(sb - 1, pending, 2 * hc, 4,
                                        on_act=False)
                        emit_evac_chunk(sb - 1, pending, 2 * hc + 1, 4,
                                        on_act=False)
                pending = pos
            # drain: last superblock in fine chunks, alternating engines
            for k in range(8):
                emit_evac_chunk(NSB - 1, pending, k, 8, on_act=(k % 2 == 1))

    nc.compile()
    return nc


def kernel(encoder_state, decoder_state, W1, b1, W2, b2):
    from concourse.bass_utils import run_bass_kernel_spmd
    global LAST_RESULT

    if "nc" not in _CACHE:
        _CACHE["nc"] = _build_program()
    nc = _CACHE["nc"]

    encoder_state = np.asarray(encoder_state, dtype=np.float32)
    decoder_state = np.asarray(decoder_state, dtype=np.float32)
    W1 = np.asarray(W1, dtype=np.float32)
    b1 = np.asarray(b1, dtype=np.float32)
    W2 = np.asarray(W2, dtype=np.float32)
    b2 = np.asarray(b2, dtype=np.float32)

    h16 = np.float16
    W1h = W1.astype(h16)
    W2h = W2.astype(h16)
    b1r = np.ascontiguousarray(b1.reshape(HC, 128).T)  # [128, 8]
    b2c = np.ascontiguousarray(b2.reshape(O, 1))

    in_maps = []
    for i in range(NCORES):
        in_maps.append({
            "encT": np.ascontiguousarray(encoder_state[i].T.astype(h16)),
            "decT": np.ascontiguousarray(decoder_state[i].T.astype(h16)),
            "W1": W1h,
            "W2h": W2h,
            "b1r": b1r,
            "b2c": b2c,
        })

    trace = bool(int(os.environ.get("KERNEL_TRACE", "0")))
    res = run_bass_kernel_spmd(nc, in_maps, list(range(NCORES)), trace=trace)
    LAST_RESULT = res

    out = np.empty((B, T, U, O), dtype=np.float32)
    for i in range(NCORES):
        out[i] = res.results[i]["outT"].transpose(2, 1, 0)
    return out
